# revision 9
# baseline (speedup 1.0000x reference)
"""Trainium2 Bass kernel for the sparse_attention nn module.

Sharding: 8 cores = 4 batches x 2 halves of the L=5120 attention rows.
Each core computes sigmoid-attention output rows for its (batch, half):
  - K/V projections over the full L for its batch (cheap, d=8)
  - Q pipeline (LSTM step + 2048->32 conv + bilinear grid-sample) for its
    2560 rows only
  - scores^T = K @ Q^T tiles (k on partitions), sigmoid on ACT from PSUM,
    out^T accumulated as V^T @ probs^T in PSUM over all k-tiles.
All attention matmuls run in bf16 (fp32 accumulation in PSUM); the small
pre-attention pipeline stays fp32.
"""
import sys

sys.path.insert(0, "/opt/trn_rl_repo")

import numpy as np

import concourse.bacc as bacc
import concourse.tile as tile
from concourse import mybir
from concourse.bass_utils import run_bass_kernel_spmd
from concourse.masks import make_identity

F32 = mybir.dt.float32
BF16 = mybir.dt.bfloat16
ALU = mybir.AluOpType
ACTF = mybir.ActivationFunctionType

B, T, N = 4, 20, 256
L = T * N            # 5120
HL = L // 2          # 2560 rows per core
HT = T // 2          # 10 t-steps per core
CMAP, CC = 2048, 32
NK = L // 128        # 40 k-tiles
NQ = HL // 512       # 5 q-chunks
KG = 2               # k-tiles per sigmoid group

_nc_cache = None


def _build():
    nc = bacc.Bacc()
    dt_in = {
        "xT": ([2, L], F32),
        "xh": ([2, HL], F32),
        "md": ([CMAP, 256], F32),
        "compwT": ([CMAP, CC], F32),
        "compb": ([CC], F32),
        "wiT": ([2, 4], F32),
        "woT": ([2, 4], F32),
        "wgT": ([2, 4], F32),
        "gbi": ([4, HT], F32),
        "gbo": ([4, HT], F32),
        "gbg": ([4, HT], F32),
        "vfTx": ([4, 4], F32),
        "vfTlc": ([32, 4], F32),
        "vfb": ([4, 1], F32),
        "fcT": ([4, 8], F32),
        "fcb": ([8, 1], F32),
        "kwT": ([2, 8], F32),
        "kb": ([8, T], F32),
        "vwT": ([2, 8], F32),
        "vb": ([8, T], F32),
        "fcoT": ([8, 2], F32),
        "fcob": ([2, 1], F32),
    }
    d = {k: nc.dram_tensor(k, sh, dt, kind="ExternalInput")
         for k, (sh, dt) in dt_in.items()}
    y_out = nc.dram_tensor("y", [2, HL], F32, kind="ExternalOutput")

    with tile.TileContext(nc) as tc:
        with tc.tile_pool(name="main", bufs=1) as pool, \
             tc.tile_pool(name="work", bufs=3) as work, \
             tc.tile_pool(name="work2", bufs=2) as work2, \
             tc.tile_pool(name="ps", bufs=2, space="PSUM") as psp, \
             tc.tile_pool(name="po", bufs=2, space="PSUM") as pop:

            # ---- persistent SBUF ----
            sb_xT = pool.tile([2, L], F32)
            nc.sync.dma_start(sb_xT, d["xT"].ap())
            sb_xhT = pool.tile([2, HL], F32)
            nc.sync.dma_start(sb_xhT, d["xh"].ap())
            # point-major raw half coords: [p, chunk(20), ch]
            x_pm = pool.tile([128, 20, 2], F32)
            for ch in range(2):
                nc.sync.dma_start(
                    x_pm[:, :, ch],
                    d["xh"].ap()[ch].rearrange("(k p) -> p k", p=128))
            sb_compwT = pool.tile([128, 16, CC], F32)
            nc.sync.dma_start(
                sb_compwT, d["compwT"].ap().rearrange("(k p) o -> p k o", p=128))
            # comp_b broadcast across partitions (DRAM source bcast)
            sb_compb = pool.tile([128, CC], F32)
            nc.sync.dma_start(sb_compb, d["compb"].ap()[None, :].to_broadcast((128, CC)))
            sml = {}
            for k in ("wiT", "woT", "wgT", "gbi", "gbo", "gbg", "vfTx", "vfTlc", "vfb", "fcT", "fcb", "kwT", "kb",
                      "vwT", "vb", "fcoT", "fcob"):
                sml[k] = pool.tile(list(d[k].shape), F32, name=k)
                nc.sync.dma_start(sml[k], d[k].ap())

            ident = pool.tile([128, 128], BF16)
            make_identity(nc, ident)
            identf = pool.tile([128, 128], F32)
            make_identity(nc, identf)
            iota16 = pool.tile([128, 16], F32)
            nc.gpsimd.iota(iota16, [[1, 16]], base=0, channel_multiplier=0,
                           allow_small_or_imprecise_dtypes=True)

            # ---- compressed feature map, transposed: cmT[spatial, ch] ----
            sb_cmT = pool.tile([128, 2, CC], F32)
            for h in range(2):
                ps_cm = psp.tile([128, CC], F32, tag="tmp")
                for k in range(16):
                    mdt = work.tile([128, 128], F32, tag="mdt")
                    nc.sync.dma_start(
                        mdt, d["md"].ap()[k * 128:(k + 1) * 128,
                                          h * 128:(h + 1) * 128])
                    nc.tensor.matmul(ps_cm, lhsT=mdt, rhs=sb_compwT[:, k, :],
                                     start=(k == 0), stop=(k == 15))
                nc.vector.tensor_tensor(sb_cmT[:, h, :], ps_cm, sb_compb, ALU.add)

            # ---- grid-sample weights (per-point scalars, big-tile DVE) ----
            ixy = pool.tile([128, 20, 2], F32)
            nc.vector.tensor_scalar(ixy, x_pm, 1.0 / 32.0, 0.5, ALU.mult, ALU.add)
            ti = pool.tile([128, 20, 2], mybir.dt.int32)
            nc.vector.tensor_copy(ti, ixy)
            tf = pool.tile([128, 20, 2], F32)
            nc.vector.tensor_copy(tf, ti)
            gt = pool.tile([128, 20, 2], F32)
            nc.vector.tensor_tensor(gt, tf, ixy, ALU.is_gt)
            x0f = pool.tile([128, 20, 2], F32)   # = floor coord + 1, in [0,16]
            nc.vector.tensor_tensor(x0f, tf, gt, ALU.subtract)
            fr = pool.tile([128, 20, 2], F32)
            nc.vector.tensor_tensor(fr, ixy, x0f, ALU.subtract)
            w0 = pool.tile([128, 20, 2], F32)
            nc.vector.tensor_scalar(w0, fr, -1.0, 1.0, ALU.mult, ALU.add)
            v0 = pool.tile([128, 20, 2], F32)
            nc.vector.tensor_scalar(v0, x0f, 0.5, None, ALU.is_ge)
            v1 = pool.tile([128, 20, 2], F32)
            nc.vector.tensor_scalar(v1, x0f, 15.5, None, ALU.is_le)
            w0e = pool.tile([128, 20, 2], F32)
            nc.vector.tensor_tensor(w0e, w0, v0, ALU.mult)
            w1e = pool.tile([128, 20, 2], F32)
            nc.vector.tensor_tensor(w1e, fr, v1, ALU.mult)
            x0c = pool.tile([128, 20, 2], F32)
            nc.vector.tensor_scalar(x0c, x0f, -1.0, 0.0, ALU.add, ALU.max)
            x1c = pool.tile([128, 20, 2], F32)
            nc.vector.tensor_scalar(x1c, x0f, 15.0, None, ALU.min)

            # one-hot corner weights Ox, Oy: [p, chunk, 16]
            ohs = []
            for a in range(2):  # 0=x, 1=y
                o_t = pool.tile([128, 20, 16], F32, name=f"oh{a}")
                tmp = pool.tile([128, 20, 16], F32, name=f"ohtmp{a}")
                nc.vector.tensor_tensor(
                    o_t, iota16[:, None, :].to_broadcast((128, 20, 16)),
                    x0c[:, :, a:a + 1].to_broadcast((128, 20, 16)), ALU.is_equal)
                nc.vector.tensor_tensor(
                    o_t, o_t, w0e[:, :, a:a + 1].to_broadcast((128, 20, 16)),
                    ALU.mult)
                nc.vector.tensor_tensor(
                    tmp, iota16[:, None, :].to_broadcast((128, 20, 16)),
                    x1c[:, :, a:a + 1].to_broadcast((128, 20, 16)), ALU.is_equal)
                nc.vector.tensor_tensor(
                    tmp, tmp, w1e[:, :, a:a + 1].to_broadcast((128, 20, 16)),
                    ALU.mult)
                nc.vector.tensor_tensor(o_t, o_t, tmp, ALU.add)
                ohs.append(o_t)
            Ox, Oy = ohs

            # Wg[p, (py,px)] = Oy*Ox outer product; PE-transpose to WgT halves
            sb_WgT = [pool.tile([128, HL], F32, name=f"wgT{h}") for h in range(2)]
            for c in range(20):
                wg = work.tile([128, 16, 16], F32, tag="wg")
                nc.vector.tensor_tensor(
                    wg, Oy[:, c, :, None].to_broadcast((128, 16, 16)),
                    Ox[:, c, None, :].to_broadcast((128, 16, 16)), ALU.mult)
                wgf = wg.rearrange("p a b -> p (a b)")
                for h in range(2):
                    ps_t = psp.tile([128, 128], F32, tag="tmp")
                    nc.tensor.transpose(ps_t, wgf[:, h * 128:(h + 1) * 128], identf)
                    nc.vector.tensor_copy(sb_WgT[h][:, c * 128:(c + 1) * 128], ps_t)

            # ---- local context lcT[ch, pts] ----
            sb_lc = pool.tile([CC, HL], F32)
            for qc in range(NQ):
                ps_lc = psp.tile([CC, 512], F32, tag="tmp")
                for h in range(2):
                    nc.tensor.matmul(
                        ps_lc, lhsT=sb_cmT[:, h, :],
                        rhs=sb_WgT[h][:, qc * 512:(qc + 1) * 512],
                        start=(h == 0), stop=(h == 1))
                nc.scalar.copy(sb_lc[:, qc * 512:(qc + 1) * 512], ps_lc)

            # ---- LSTM gates: separate i/o/g tensors (ACT needs base part 0)
            g_i = pool.tile([4, HL], F32)
            g_o = pool.tile([4, HL], F32)
            g_g = pool.tile([4, HL], F32)
            for t in range(HT):
                sl = slice(t * 256, (t + 1) * 256)
                for gt, wk, bk in ((g_i, "wiT", "gbi"), (g_o, "woT", "gbo"),
                                   (g_g, "wgT", "gbg")):
                    ps_g = psp.tile([4, 256], F32, tag="tmp", name="ps_g")
                    nc.tensor.matmul(ps_g, lhsT=sml[wk], rhs=sb_xhT[:, sl])
                    nc.vector.tensor_scalar(gt[:, sl], ps_g,
                                            sml[bk][:, t:t + 1], None, ALU.add)
            nc.scalar.activation(g_i, g_i, ACTF.Sigmoid)
            nc.scalar.activation(g_o, g_o, ACTF.Sigmoid)
            nc.scalar.activation(g_g, g_g, ACTF.Tanh)
            c_t = pool.tile([4, HL], F32)
            nc.vector.tensor_tensor(c_t, g_i, g_g, ALU.mult)
            nc.scalar.activation(c_t, c_t, ACTF.Tanh)
            XT = g_g
            nc.vector.tensor_tensor(XT, g_o, c_t, ALU.mult)

            # ---- X2 = vf([X; lc]), Q = fc(X2) ----
            sb_QT = pool.tile([8, HL], BF16)
            for qc in range(NQ):
                sl = slice(qc * 512, (qc + 1) * 512)
                ps_x2 = psp.tile([4, 512], F32, tag="tmp")
                nc.tensor.matmul(ps_x2, lhsT=sml["vfTx"], rhs=XT[:, sl],
                                 start=True, stop=False)
                nc.tensor.matmul(ps_x2, lhsT=sml["vfTlc"], rhs=sb_lc[:, sl],
                                 start=False, stop=True)
                x2sb = work2.tile([4, 512], F32, tag="x2")
                nc.vector.tensor_scalar(x2sb, ps_x2, sml["vfb"], None, ALU.add)
                ps_q = psp.tile([8, 512], F32, tag="tmp")
                nc.tensor.matmul(ps_q, lhsT=sml["fcT"], rhs=x2sb)
                nc.vector.tensor_scalar(sb_QT[:, sl], ps_q, sml["fcb"], None,
                                        ALU.add)

            # ---- K, V over full L ----
            sb_KT = pool.tile([8, L], BF16)
            sb_VT = pool.tile([8, L], BF16)
            for t in range(T):
                sl = slice(t * 256, (t + 1) * 256)
                ps_k = psp.tile([8, 256], F32, tag="tmp")
                nc.tensor.matmul(ps_k, lhsT=sml["kwT"], rhs=sb_xT[:, sl])
                nc.vector.tensor_scalar(sb_KT[:, sl], ps_k,
                                        sml["kb"][:, t:t + 1], None, ALU.add)
                ps_v = psp.tile([8, 256], F32, tag="tmp")
                nc.tensor.matmul(ps_v, lhsT=sml["vwT"], rhs=sb_xT[:, sl])
                nc.vector.tensor_scalar(sb_VT[:, sl], ps_v,
                                        sml["vb"][:, t:t + 1], None, ALU.add)
            # V into (k,d) layout via PE transposes
            sb_Vkd = pool.tile([128, NK, 8], BF16)
            for ki in range(NK):
                ps_vt = psp.tile([128, 8], BF16, tag="tmp")
                nc.tensor.transpose(ps_vt, sb_VT[:, ki * 128:(ki + 1) * 128],
                                    ident[0:8, 0:8])
                nc.vector.tensor_copy(sb_Vkd[:, ki, :], ps_vt)

            # ---- attention ----
            sb_y = pool.tile([2, HL], F32)
            for qc in range(NQ):
                qsl = slice(qc * 512, (qc + 1) * 512)
                ps_o = pop.tile([8, 512], F32, tag="po")
                for kg in range(NK // KG):
                    ps_s = psp.tile([128, KG * 512], F32, tag="scores")
                    for j in range(KG):
                        ki = kg * KG + j
                        nc.tensor.matmul(
                            ps_s[:, j * 512:(j + 1) * 512],
                            lhsT=sb_KT[:, ki * 128:(ki + 1) * 128],
                            rhs=sb_QT[:, qsl], start=True, stop=True)
                    probs = work.tile([128, KG * 512], BF16, tag="probs")
                    nc.scalar.activation(probs, ps_s, ACTF.Sigmoid)
                    for j in range(KG):
                        ki = kg * KG + j
                        nc.tensor.matmul(
                            ps_o, lhsT=sb_Vkd[:, ki, :],
                            rhs=probs[:, j * 512:(j + 1) * 512],
                            start=(ki == 0), stop=(ki == NK - 1))
                # epilogue: threshold relu + final projection
                msk = work2.tile([8, 512], F32, tag="msk")
                nc.vector.tensor_scalar(msk, ps_o, 0.5, None, ALU.is_gt)
                oT = work2.tile([8, 512], F32, tag="ot")
                nc.vector.tensor_tensor(oT, ps_o, msk, ALU.mult)
                ps_y = psp.tile([2, 512], F32, tag="tmp")
                nc.tensor.matmul(ps_y, lhsT=sml["fcoT"], rhs=oT)
                nc.vector.tensor_scalar(sb_y[:, qsl], ps_y, sml["fcob"], None,
                                        ALU.add)
            nc.sync.dma_start(y_out.ap(), sb_y)

    nc.compile()
    return nc


def _prep_inputs(x, metadata, w_ih, b_ih, b_hh, comp_w, comp_b, vf_w, vf_b,
                 fc_w, fc_b, fc2_w, fc2_b, fc3_w, fc3_b, fco_w, fco_b):
    f = np.float32
    pos = np.arange(T, dtype=f)
    pe = np.stack([np.sin(pos), np.cos(pos)], axis=-1).astype(f)  # (T,2)
    w_ih = np.asarray(w_ih, f)
    bb = np.asarray(b_ih, f) + np.asarray(b_hh, f)
    w_i, w_g, w_o = w_ih[0:4], w_ih[8:12], w_ih[12:16]
    gb_i = (pe @ w_i.T + bb[0:4]).T
    gb_g = (pe @ w_g.T + bb[8:12]).T
    gb_o = (pe @ w_o.T + bb[12:16]).T
    kb = (pe @ np.asarray(fc2_w, f).T + np.asarray(fc2_b, f)).T  # (8,T)
    vb = (pe @ np.asarray(fc3_w, f).T + np.asarray(fc3_b, f)).T
    common = dict(
        compwT=np.ascontiguousarray(np.asarray(comp_w, f).T),
        compb=np.asarray(comp_b, f),
        wiT=np.ascontiguousarray(w_i.T), woT=np.ascontiguousarray(w_o.T),
        wgT=np.ascontiguousarray(w_g.T),
        vfTx=np.ascontiguousarray(np.asarray(vf_w, f).T[0:4]),
        vfTlc=np.ascontiguousarray(np.asarray(vf_w, f).T[4:36]),
        vfb=np.asarray(vf_b, f).reshape(4, 1),
        fcT=np.ascontiguousarray(np.asarray(fc_w, f).T),
        fcb=np.asarray(fc_b, f).reshape(8, 1),
        kwT=np.ascontiguousarray(np.asarray(fc2_w, f).T),
        kb=np.ascontiguousarray(kb),
        vwT=np.ascontiguousarray(np.asarray(fc3_w, f).T),
        vb=np.ascontiguousarray(vb),
        fcoT=np.ascontiguousarray(np.asarray(fco_w, f).T),
        fcob=np.asarray(fco_b, f).reshape(2, 1),
    )
    in_maps = []
    for core in range(8):
        b_, hi = core // 2, core % 2
        xb = np.ascontiguousarray(np.asarray(x[b_], f).reshape(2, L))
        m = dict(common)
        m["xT"] = xb
        m["xh"] = np.ascontiguousarray(xb[:, hi * HL:(hi + 1) * HL])
        m["md"] = np.ascontiguousarray(
            np.asarray(metadata[b_], f).reshape(CMAP, 256))
        m["gbi"] = np.ascontiguousarray(gb_i[:, hi * HT:(hi + 1) * HT])
        m["gbo"] = np.ascontiguousarray(gb_o[:, hi * HT:(hi + 1) * HT])
        m["gbg"] = np.ascontiguousarray(gb_g[:, hi * HT:(hi + 1) * HT])
        in_maps.append(m)
    return in_maps


def kernel(**inputs):
    global _nc_cache
    if _nc_cache is None:
        _nc_cache = _build()
    in_maps = _prep_inputs(**inputs)
    res = run_bass_kernel_spmd(_nc_cache, in_maps, core_ids=list(range(8)))
    out = np.zeros((B, 2, T, N), np.float32)
    for core in range(8):
        b_, hi = core // 2, core % 2
        y = np.asarray(res.results[core]["y"]).reshape(2, HT, N)
        out[b_, :, hi * HT:(hi + 1) * HT, :] = y
    return out


# revision 12
# speedup vs baseline: 1.1832x; 1.1832x over previous
"""Trainium2 Bass kernel for the sparse_attention nn module.

Sharding: 8 cores = 4 batches x 2 halves of the L=5120 attention rows.
Each core computes sigmoid-attention output rows for its (batch, half):
  - K/V projections over the full L for its batch (cheap, d=8)
  - Q pipeline (LSTM step + 2048->32 conv + bilinear grid-sample) for its
    2560 rows only
  - scores^T = K @ Q^T tiles (k on partitions), sigmoid on ACT from PSUM,
    out^T accumulated as V^T @ probs^T in PSUM over all k-tiles.
All attention matmuls run in bf16 (fp32 accumulation in PSUM); the small
pre-attention pipeline stays fp32.
"""
import sys

sys.path.insert(0, "/opt/trn_rl_repo")

import numpy as np

import concourse.bacc as bacc
import concourse.tile as tile
from concourse import mybir
from concourse.bass_utils import run_bass_kernel_spmd
from concourse.masks import make_identity

F32 = mybir.dt.float32
BF16 = mybir.dt.bfloat16
ALU = mybir.AluOpType
ACTF = mybir.ActivationFunctionType

B, T, N = 4, 20, 256
L = T * N            # 5120
HL = L // 2          # 2560 rows per core
HT = T // 2          # 10 t-steps per core
CMAP, CC = 2048, 32
NK = L // 128        # 40 k-tiles
NQ = HL // 512       # 5 q-chunks
KG = 2               # k-tiles per sigmoid group

_nc_cache = None


def _build():
    nc = bacc.Bacc()
    dt_in = {
        "xT": ([2, L], F32),
        "xh": ([2, HL], F32),
        "md": ([CMAP, 256], F32),
        "compwT": ([CMAP, CC], F32),
        "compb": ([CC, 1], F32),
        "wiT": ([2, 4], F32),
        "woT": ([2, 4], F32),
        "wgT": ([2, 4], F32),
        "gbi": ([4, HT], F32),
        "gbo": ([4, HT], F32),
        "gbg": ([4, HT], F32),
        "vfTx": ([4, 4], F32),
        "vfTlc": ([32, 4], F32),
        "vfb": ([4, 1], F32),
        "fcT": ([4, 8], F32),
        "fcb": ([8, 1], F32),
        "kwT": ([2, 8], F32),
        "kb": ([8, T], F32),
        "vwT": ([2, 8], F32),
        "vb": ([8, T], F32),
        "fcoT": ([8, 2], F32),
        "fcob": ([2, 1], F32),
    }
    d = {k: nc.dram_tensor(k, sh, dt, kind="ExternalInput")
         for k, (sh, dt) in dt_in.items()}
    y_out = nc.dram_tensor("y", [2, HL], F32, kind="ExternalOutput")

    with tile.TileContext(nc) as tc:
        with tc.tile_pool(name="main", bufs=1) as pool, \
             tc.tile_pool(name="work", bufs=3) as work, \
             tc.tile_pool(name="work2", bufs=2) as work2, \
             tc.tile_pool(name="ps", bufs=2, space="PSUM") as psp, \
             tc.tile_pool(name="po", bufs=2, space="PSUM") as pop:

            # ---- persistent SBUF ----
            sb_xT = pool.tile([2, L], F32)
            nc.sync.dma_start(sb_xT, d["xT"].ap())
            sb_xhT = pool.tile([2, HL], F32)
            nc.sync.dma_start(sb_xhT, d["xh"].ap())
            # point-major raw half coords: [p, chunk(20), ch]
            x_pm = pool.tile([128, 20, 2], F32)
            for ch in range(2):
                nc.sync.dma_start(
                    x_pm[:, :, ch],
                    d["xh"].ap()[ch].rearrange("(k p) -> p k", p=128))
            sb_compwT = pool.tile([128, 16, CC], F32)
            nc.sync.dma_start(
                sb_compwT, d["compwT"].ap().rearrange("(k p) o -> p k o", p=128))
            sml = {}
            for k in ("compb", "wiT", "woT", "wgT", "gbi", "gbo", "gbg", "vfTx", "vfTlc", "vfb", "fcT", "fcb", "kwT", "kb",
                      "vwT", "vb", "fcoT", "fcob"):
                sml[k] = pool.tile(list(d[k].shape), F32, name=k)
                nc.sync.dma_start(sml[k], d[k].ap())

            ident = pool.tile([128, 128], BF16)
            make_identity(nc, ident)
            identf = pool.tile([128, 128], F32)
            make_identity(nc, identf)
            iota16 = pool.tile([128, 16], F32)
            nc.gpsimd.iota(iota16, [[1, 16]], base=0, channel_multiplier=0,
                           allow_small_or_imprecise_dtypes=True)

            # ---- compressed feature map cm[ch, spatial] then cmT ----
            sb_cmT = pool.tile([128, 2, CC], F32)
            ps_cm = psp.tile([CC, 256], F32, tag="tmp")
            for k in range(16):
                mdt = work.tile([128, 256], F32, tag="mdt")
                nc.sync.dma_start(mdt, d["md"].ap()[k * 128:(k + 1) * 128, :])
                nc.tensor.matmul(ps_cm, lhsT=sb_compwT[:, k, :], rhs=mdt,
                                 start=(k == 0), stop=(k == 15))
            sb_cm = pool.tile([CC, 256], F32)
            nc.vector.tensor_scalar(sb_cm, ps_cm, sml["compb"], None, ALU.add)
            for h in range(2):
                ps_ct = psp.tile([128, CC], F32, tag="tmp")
                nc.tensor.transpose(ps_ct, sb_cm[:, h * 128:(h + 1) * 128],
                                    identf[0:CC, 0:CC])
                nc.vector.tensor_copy(sb_cmT[:, h, :], ps_ct)

            # ---- grid-sample weights (per-point scalars, big-tile DVE) ----
            ixy = pool.tile([128, 20, 2], F32)
            nc.vector.tensor_scalar(ixy, x_pm, 1.0 / 32.0, 0.5, ALU.mult, ALU.add)
            ti = pool.tile([128, 20, 2], mybir.dt.int32)
            nc.vector.tensor_copy(ti, ixy)
            tf = pool.tile([128, 20, 2], F32)
            nc.vector.tensor_copy(tf, ti)
            gt = pool.tile([128, 20, 2], F32)
            nc.vector.tensor_tensor(gt, tf, ixy, ALU.is_gt)
            x0f = pool.tile([128, 20, 2], F32)   # = floor coord + 1, in [0,16]
            nc.vector.tensor_tensor(x0f, tf, gt, ALU.subtract)
            fr = pool.tile([128, 20, 2], F32)
            nc.vector.tensor_tensor(fr, ixy, x0f, ALU.subtract)
            w0 = pool.tile([128, 20, 2], F32)
            nc.vector.tensor_scalar(w0, fr, -1.0, 1.0, ALU.mult, ALU.add)
            v0 = pool.tile([128, 20, 2], F32)
            nc.vector.tensor_scalar(v0, x0f, 0.5, None, ALU.is_ge)
            v1 = pool.tile([128, 20, 2], F32)
            nc.vector.tensor_scalar(v1, x0f, 15.5, None, ALU.is_le)
            w0e = pool.tile([128, 20, 2], F32)
            nc.vector.tensor_tensor(w0e, w0, v0, ALU.mult)
            w1e = pool.tile([128, 20, 2], F32)
            nc.vector.tensor_tensor(w1e, fr, v1, ALU.mult)
            x0c = pool.tile([128, 20, 2], F32)
            nc.vector.tensor_scalar(x0c, x0f, -1.0, 0.0, ALU.add, ALU.max)
            x1c = pool.tile([128, 20, 2], F32)
            nc.vector.tensor_scalar(x1c, x0f, 15.0, None, ALU.min)

            # one-hot corner weights Ox, Oy: [p, chunk, 16]
            ohs = []
            for a in range(2):  # 0=x, 1=y
                o_t = pool.tile([128, 20, 16], F32, name=f"oh{a}")
                tmp = pool.tile([128, 20, 16], F32, name=f"ohtmp{a}")
                nc.vector.tensor_tensor(
                    o_t, iota16[:, None, :].to_broadcast((128, 20, 16)),
                    x0c[:, :, a:a + 1].to_broadcast((128, 20, 16)), ALU.is_equal)
                nc.vector.tensor_tensor(
                    o_t, o_t, w0e[:, :, a:a + 1].to_broadcast((128, 20, 16)),
                    ALU.mult)
                nc.vector.tensor_tensor(
                    tmp, iota16[:, None, :].to_broadcast((128, 20, 16)),
                    x1c[:, :, a:a + 1].to_broadcast((128, 20, 16)), ALU.is_equal)
                nc.vector.tensor_tensor(
                    tmp, tmp, w1e[:, :, a:a + 1].to_broadcast((128, 20, 16)),
                    ALU.mult)
                nc.vector.tensor_tensor(o_t, o_t, tmp, ALU.add)
                ohs.append(o_t)
            Ox, Oy = ohs

            # Wg[p, (py,px)] = Oy*Ox outer product; PE-transpose to WgT halves
            sb_WgT = [pool.tile([128, HL], F32, name=f"wgT{h}") for h in range(2)]
            for c in range(20):
                wg = work.tile([128, 16, 16], F32, tag="wg")
                nc.vector.tensor_tensor(
                    wg, Oy[:, c, :, None].to_broadcast((128, 16, 16)),
                    Ox[:, c, None, :].to_broadcast((128, 16, 16)), ALU.mult)
                wgf = wg.rearrange("p a b -> p (a b)")
                for h in range(2):
                    ps_t = psp.tile([128, 128], F32, tag="tmp")
                    nc.tensor.transpose(ps_t, wgf[:, h * 128:(h + 1) * 128], identf)
                    nc.vector.tensor_copy(sb_WgT[h][:, c * 128:(c + 1) * 128], ps_t)

            # ---- local context lcT[ch, pts] ----
            sb_lc = pool.tile([CC, HL], F32)
            for qc in range(NQ):
                ps_lc = psp.tile([CC, 512], F32, tag="tmp")
                for h in range(2):
                    nc.tensor.matmul(
                        ps_lc, lhsT=sb_cmT[:, h, :],
                        rhs=sb_WgT[h][:, qc * 512:(qc + 1) * 512],
                        start=(h == 0), stop=(h == 1))
                nc.scalar.copy(sb_lc[:, qc * 512:(qc + 1) * 512], ps_lc)

            # ---- LSTM gates: separate i/o/g tensors (ACT needs base part 0)
            g_i = pool.tile([4, HL], F32)
            g_o = pool.tile([4, HL], F32)
            g_g = pool.tile([4, HL], F32)
            for c in range(HL // 512):
                sl = slice(c * 512, (c + 1) * 512)
                for gt, wk, bk in ((g_i, "wiT", "gbi"), (g_o, "woT", "gbo"),
                                   (g_g, "wgT", "gbg")):
                    ps_g = psp.tile([4, 512], F32, tag="tmp", name="ps_g")
                    nc.tensor.matmul(ps_g, lhsT=sml[wk], rhs=sb_xhT[:, sl])
                    bb = sml[bk][:, 2 * c:2 * c + 2, None].to_broadcast(
                        (4, 2, 256))
                    nc.vector.tensor_tensor(
                        gt[:, sl].rearrange("p (a b) -> p a b", a=2),
                        ps_g.rearrange("p (a b) -> p a b", a=2), bb, ALU.add)
            nc.scalar.activation(g_i, g_i, ACTF.Sigmoid)
            nc.scalar.activation(g_o, g_o, ACTF.Sigmoid)
            nc.scalar.activation(g_g, g_g, ACTF.Tanh)
            c_t = pool.tile([4, HL], F32)
            nc.vector.tensor_tensor(c_t, g_i, g_g, ALU.mult)
            nc.scalar.activation(c_t, c_t, ACTF.Tanh)
            XT = g_g
            nc.vector.tensor_tensor(XT, g_o, c_t, ALU.mult)

            # ---- X2 = vf([X; lc]), Q = fc(X2) ----
            sb_QT = pool.tile([128, HL], BF16)  # rows 0-7 = Q^T, rows 32-39 copy
            for qc in range(NQ):
                sl = slice(qc * 512, (qc + 1) * 512)
                ps_x2 = psp.tile([4, 512], F32, tag="tmp")
                nc.tensor.matmul(ps_x2, lhsT=sml["vfTx"], rhs=XT[:, sl],
                                 start=True, stop=False)
                nc.tensor.matmul(ps_x2, lhsT=sml["vfTlc"], rhs=sb_lc[:, sl],
                                 start=False, stop=True)
                x2sb = work2.tile([4, 512], F32, tag="x2")
                nc.vector.tensor_scalar(x2sb, ps_x2, sml["vfb"], None, ALU.add)
                ps_q = psp.tile([8, 512], F32, tag="tmp")
                nc.tensor.matmul(ps_q, lhsT=sml["fcT"], rhs=x2sb)
                nc.vector.tensor_scalar(sb_QT[0:8, sl], ps_q, sml["fcb"], None,
                                        ALU.add)

            # ---- K, V over full L ----
            sb_KT = pool.tile([128, L], BF16)   # rows 0-7 = K^T, rows 32-39 copy
            sb_VT = pool.tile([8, L], BF16)
            for c in range(L // 512):
                sl = slice(c * 512, (c + 1) * 512)
                ps_k = psp.tile([8, 512], F32, tag="tmp")
                nc.tensor.matmul(ps_k, lhsT=sml["kwT"], rhs=sb_xT[:, sl])
                kbb = sml["kb"][:, 2 * c:2 * c + 2, None].to_broadcast(
                    (8, 2, 256))
                nc.vector.tensor_tensor(
                    sb_KT[0:8, sl].rearrange("p (a b) -> p a b", a=2),
                    ps_k.rearrange("p (a b) -> p a b", a=2), kbb, ALU.add)
                ps_v = psp.tile([8, 512], F32, tag="tmp")
                nc.tensor.matmul(ps_v, lhsT=sml["vwT"], rhs=sb_xT[:, sl])
                vbb = sml["vb"][:, 2 * c:2 * c + 2, None].to_broadcast(
                    (8, 2, 256))
                nc.vector.tensor_tensor(
                    sb_VT[:, sl].rearrange("p (a b) -> p a b", a=2),
                    ps_v.rearrange("p (a b) -> p a b", a=2), vbb, ALU.add)
            nc.sync.dma_start(sb_KT[32:40, :], sb_KT[0:8, :])
            # V into (k,d) layout via PE transposes
            sb_Vkd = pool.tile([128, NK, 8], BF16)
            for ki in range(NK):
                ps_vt = psp.tile([128, 8], BF16, tag="tmp")
                nc.tensor.transpose(ps_vt, sb_VT[:, ki * 128:(ki + 1) * 128],
                                    ident[0:8, 0:8])
                nc.vector.tensor_copy(sb_Vkd[:, ki, :], ps_vt)

            # ---- attention ----
            sb_y = pool.tile([2, HL], F32)
            qrep = nc.sync.dma_start(sb_QT[32:40, :], sb_QT[0:8, :])
            for qc in range(NQ):
                qsl = slice(qc * 512, (qc + 1) * 512)
                ps_o = pop.tile([128, 512], F32, tag="po")
                for kg in range(NK // KG):
                    ps_s = psp.tile([128, KG * 512], F32, tag="scores")
                    for j in range(KG):
                        ki = kg * KG + j
                        rg = 32 * j
                        nc.tensor.matmul(
                            ps_s[:, j * 512:(j + 1) * 512],
                            lhsT=sb_KT[rg:rg + 8, ki * 128:(ki + 1) * 128],
                            rhs=sb_QT[rg:rg + 8, qsl], start=True, stop=True,
                            tile_position=(rg, 0))
                    probs = work.tile([128, KG * 512], BF16, tag="probs")
                    nc.scalar.activation(probs, ps_s, ACTF.Sigmoid)
                    for j in range(KG):
                        ki = kg * KG + j
                        cg = 32 * (ki % 4)
                        nc.tensor.matmul(
                            ps_o[cg:cg + 8, :], lhsT=sb_Vkd[:, ki, :],
                            rhs=probs[:, j * 512:(j + 1) * 512],
                            start=(ki < 4), stop=(ki >= NK - 4),
                            tile_position=(0, cg), skip_group_check=True)
                # epilogue: reduce 4 col-group partials, threshold, project
                o01 = work2.tile([8, 512], F32, tag="o01")
                nc.vector.tensor_copy(o01, ps_o[0:8, :])
                o02 = work2.tile([8, 512], F32, tag="o02")
                nc.vector.tensor_tensor(o02, ps_o[32:40, :], o01, ALU.add)
                o03 = work2.tile([8, 512], F32, tag="o03")
                nc.vector.tensor_tensor(o03, ps_o[64:72, :], o02, ALU.add)
                oS = work2.tile([8, 512], F32, tag="oS")
                nc.vector.tensor_tensor(oS, ps_o[96:104, :], o03, ALU.add)
                msk = work2.tile([8, 512], F32, tag="msk")
                nc.vector.tensor_scalar(msk, oS, 0.5, None, ALU.is_gt)
                oT = work2.tile([8, 512], F32, tag="ot")
                nc.vector.tensor_tensor(oT, oS, msk, ALU.mult)
                ps_y = psp.tile([2, 512], F32, tag="tmp")
                nc.tensor.matmul(ps_y, lhsT=sml["fcoT"], rhs=oT)
                nc.vector.tensor_scalar(sb_y[:, qsl], ps_y, sml["fcob"], None,
                                        ALU.add)
            nc.sync.dma_start(y_out.ap(), sb_y)

    nc.compile()
    return nc


def _prep_inputs(x, metadata, w_ih, b_ih, b_hh, comp_w, comp_b, vf_w, vf_b,
                 fc_w, fc_b, fc2_w, fc2_b, fc3_w, fc3_b, fco_w, fco_b):
    f = np.float32
    pos = np.arange(T, dtype=f)
    pe = np.stack([np.sin(pos), np.cos(pos)], axis=-1).astype(f)  # (T,2)
    w_ih = np.asarray(w_ih, f)
    bb = np.asarray(b_ih, f) + np.asarray(b_hh, f)
    w_i, w_g, w_o = w_ih[0:4], w_ih[8:12], w_ih[12:16]
    gb_i = (pe @ w_i.T + bb[0:4]).T
    gb_g = (pe @ w_g.T + bb[8:12]).T
    gb_o = (pe @ w_o.T + bb[12:16]).T
    kb = (pe @ np.asarray(fc2_w, f).T + np.asarray(fc2_b, f)).T  # (8,T)
    vb = (pe @ np.asarray(fc3_w, f).T + np.asarray(fc3_b, f)).T
    common = dict(
        compwT=np.ascontiguousarray(np.asarray(comp_w, f).T),
        compb=np.asarray(comp_b, f).reshape(CC, 1),
        wiT=np.ascontiguousarray(w_i.T), woT=np.ascontiguousarray(w_o.T),
        wgT=np.ascontiguousarray(w_g.T),
        vfTx=np.ascontiguousarray(np.asarray(vf_w, f).T[0:4]),
        vfTlc=np.ascontiguousarray(np.asarray(vf_w, f).T[4:36]),
        vfb=np.asarray(vf_b, f).reshape(4, 1),
        fcT=np.ascontiguousarray(np.asarray(fc_w, f).T),
        fcb=np.asarray(fc_b, f).reshape(8, 1),
        kwT=np.ascontiguousarray(np.asarray(fc2_w, f).T),
        kb=np.ascontiguousarray(kb),
        vwT=np.ascontiguousarray(np.asarray(fc3_w, f).T),
        vb=np.ascontiguousarray(vb),
        fcoT=np.ascontiguousarray(np.asarray(fco_w, f).T),
        fcob=np.asarray(fco_b, f).reshape(2, 1),
    )
    in_maps = []
    for core in range(8):
        b_, hi = core // 2, core % 2
        xb = np.ascontiguousarray(np.asarray(x[b_], f).reshape(2, L))
        m = dict(common)
        m["xT"] = xb
        m["xh"] = np.ascontiguousarray(xb[:, hi * HL:(hi + 1) * HL])
        m["md"] = np.ascontiguousarray(
            np.asarray(metadata[b_], f).reshape(CMAP, 256))
        m["gbi"] = np.ascontiguousarray(gb_i[:, hi * HT:(hi + 1) * HT])
        m["gbo"] = np.ascontiguousarray(gb_o[:, hi * HT:(hi + 1) * HT])
        m["gbg"] = np.ascontiguousarray(gb_g[:, hi * HT:(hi + 1) * HT])
        in_maps.append(m)
    return in_maps


def kernel(**inputs):
    global _nc_cache
    if _nc_cache is None:
        _nc_cache = _build()
    in_maps = _prep_inputs(**inputs)
    res = run_bass_kernel_spmd(_nc_cache, in_maps, core_ids=list(range(8)))
    out = np.zeros((B, 2, T, N), np.float32)
    for core in range(8):
        b_, hi = core // 2, core % 2
        y = np.asarray(res.results[core]["y"]).reshape(2, HT, N)
        out[b_, :, hi * HT:(hi + 1) * HT, :] = y
    return out
